# revision 2
# baseline (speedup 1.0000x reference)
"""Trainium2 Bass kernel for nn_FLinemodel_37185826849029 (bf16 rewrite).

Model (per batch b):
  Q = x@wq, K = x@wk, V = x@wv          [S,4]
  L = (Q K^T) @ W_at + b_at             [S,S]   <- rewritten as Q @ (K^T W_at)
  A = softmax(L, axis=-1)
  y = A @ V                             [S,4]
  p = softmax(y @ w_cls + b_cls)        [S,10]
  out = mean_s p                        [10]

vs the fp32 baseline:
  - x and w_at are cast to bf16 on the host: all large matmuls run at
    1 cyc/row instead of 4, and HBM traffic halves.
  - x^T is produced by two xbar DMA-transposes straight from DRAM
    (2-byte dtype makes this legal), deleting the PE-transpose +
    PSUM-copy machinery of stage 1.
  - natural s/u ordering everywhere (s = 128*q + p); W_at loads as
    [p, q, u] strided rows.
  - stage 3 pipelines PE (l, y matmuls) against ACT (exp) with two
    half-batch l/e buffers; PSUM = 4 banks y + 2*2 banks l.
Sharding: data-parallel over batch, 4 batches per core.
"""

from contextlib import ExitStack

import numpy as np

import concourse.bacc as bacc
import concourse.mybir as mybir
import concourse.tile as tile
from concourse import masks

F32 = mybir.dt.float32
BF16 = mybir.dt.bfloat16
EXP = mybir.ActivationFunctionType.Exp
P = 128

B, S_FULL, H_FULL, D, C = 32, 2048, 256, 4, 10
N_CORES = 8
B_LOC = B // N_CORES


def build_nc(b_loc=B_LOC, s=S_FULL, h=H_FULL, reps=1, dbg=None):
    HC = h // P            # h chunks (2)
    R = s // P             # u tiles of 128 (16)
    SC = s // 512          # 512-wide s blocks (4)
    E = C + 1
    assert HC == 2 and s % 512 == 0

    nc = bacc.Bacc("TRN2", debug=False, target_bir_lowering=False)

    xs_t = nc.dram_tensor("xs", [b_loc, s, h], BF16, kind="ExternalInput")
    wq_t = nc.dram_tensor("wq", [h, D], F32, kind="ExternalInput")
    wk_t = nc.dram_tensor("wk", [h, D], F32, kind="ExternalInput")
    wv_t = nc.dram_tensor("wv", [h, D], F32, kind="ExternalInput")
    wat_t = nc.dram_tensor("w_at", [s, s], BF16, kind="ExternalInput")
    bat_t = nc.dram_tensor("b_at", [s], F32, kind="ExternalInput")
    wcls_t = nc.dram_tensor("w_cls", [D, C], F32, kind="ExternalInput")
    bcls_t = nc.dram_tensor("b_cls", [C], F32, kind="ExternalInput")
    out_t = nc.dram_tensor("out", [b_loc, C], F32, kind="ExternalOutput")

    xs, wat = xs_t.ap(), wat_t.ap()

    with ExitStack() as ctx:
        tc = ctx.enter_context(tile.TileContext(nc))
        const = ctx.enter_context(tc.tile_pool(name="const", bufs=1))
        big = ctx.enter_context(tc.tile_pool(name="big", bufs=1))

        ident = const.tile([P, P], F32)
        masks.make_identity(nc, ident[:])
        ones_col = const.tile([P, 1], F32)
        nc.vector.memset(ones_col[:], 1.0)

        # w{q,kv}_sb[p, hc, d] = w[hc*P + p, d]; loaded flat (1 descriptor each)
        # and redistributed across partitions with tiny PE transposes.
        wq_sb = const.tile([P, HC, D], BF16)
        wkv_sb = const.tile([P, HC, 2 * D], BF16)
        wflat = const.tile([1, 3, h * D], F32)
        nc.sync.dma_start(wflat[:, 0], wq_t.ap().rearrange("h d -> (h d)")[None, :])
        nc.sync.dma_start(wflat[:, 1], wk_t.ap().rearrange("h d -> (h d)")[None, :])
        nc.sync.dma_start(wflat[:, 2], wv_t.ap().rearrange("h d -> (h d)")[None, :])

        # classifier weights extended: rows (d0..d3, bias), cols (c0..c9, unit),
        # replicated at partition strips 32b for the row-tiled z matmuls.
        wce = const.tile([P, b_loc * E], F32)
        nc.vector.memset(wce[:], 0.0)
        for b in range(b_loc):
            nc.sync.dma_start(wce[32 * b : 32 * b + D, E * b : E * b + C], wcls_t.ap())
            nc.sync.dma_start(
                wce[32 * b + D : 32 * b + D + 1, E * b : E * b + C], bcls_t.ap()[None, :])
            # unit entry at (row D, col E*b + C) of strip b
            nc.gpsimd.affine_select(
                out=wce[32 * b : 32 * b + 32, :],
                in_=wce[32 * b : 32 * b + 32, :],
                pattern=[[1, b_loc * E]],
                compare_op=mybir.AluOpType.not_equal,
                fill=1.0,
                base=-(b_loc * E * D + E * b + C),
                channel_multiplier=b_loc * E,
            )

        with tc.tile_pool(name="ps_w", bufs=1, space="PSUM") as ps_w:
            wtp = ps_w.tile([P, 3 * HC * D], F32)
            wfv = wflat[:].rearrange("o w (hh hl d) -> o w hh d hl", hh=HC, d=D)
            for w in range(3):
                for hc in range(HC):
                    for d in range(D):
                        nc.tensor.transpose(
                            wtp[:, (w * HC + hc) * D + d : (w * HC + hc) * D + d + 1],
                            wfv[:, w, hc, d, :],
                            ident[0:1, 0:1],
                        )
            nc.vector.tensor_copy(
                wq_sb[:], wtp[:, : HC * D].rearrange("p (hh d) -> p hh d", d=D))
            nc.vector.tensor_copy(
                wkv_sb[:].rearrange("p hh (w d) -> p w hh d", d=D),
                wtp[:, HC * D :].rearrange("p (w hh d) -> p w hh d", hh=HC, d=D),
            )

        xT = big.tile([P, HC, b_loc, s], BF16)      # x^T: [h-half, b, s] natural s
        wg = big.tile([P, R, s], BF16)              # W_at[128q+p, u]
        bat_sb = big.tile([P, R], F32)              # b_at[128rt + p]
        qt_sb = big.tile([P, s], BF16)              # rows 32b..: Q_b^T [4, s]
        k_pad = big.tile([P, R, P], BF16)           # col 32b+d = K_b[128q+p, d], else 0
        v_sb = big.tile([P, b_loc, R, D + 1], BF16)  # V[128rt+p, d] + ones col
        m_sb = big.tile([P, s], BF16)               # rows 32b..: M^T, col u natural
        yt_sb = big.tile([P, s], F32)               # rows 32b..: [yhat^T; rowsum]

        nc.vector.memset(v_sb[:], 1.0)
        nc.vector.memset(k_pad[:], 0.0)

        if dbg is not None:
            dbg.update(xT=xT, wg=wg, bat_sb=bat_sb, qt_sb=qt_sb, k_pad=k_pad,
                       v_sb=v_sb, m_sb=m_sb, yt_sb=yt_sb, wq_sb=wq_sb,
                       wkv_sb=wkv_sb, wce=wce)

        for _rep in range(reps):
            # ---- stage 0: all input DMAs ----
            # W_at by strided rows: partition p holds rows {128q+p}.
            nc.sync.dma_start(
                wg[:], wat.rearrange("(q p) u -> p q u", p=P))
            # x^T via xbar DMA-transpose, one call per h-half.
            for hc in range(HC):
                nc.sync.dma_start(
                    xT[:, hc].rearrange("p b s -> p (b s)"),
                    xs.rearrange("b s h -> (b s) h")[:, hc * P : (hc + 1) * P],
                    transpose=True,
                )
            # b_at: load [q, p] then PE-transpose to [p, q].
            with tc.tile_pool(name="batld", bufs=1) as batld, \
                 tc.tile_pool(name="ps_bat", bufs=1, space="PSUM") as ps_bat:
                bflat = batld.tile([R, P], F32)
                nc.sync.dma_start(bflat[:], bat_t.ap().rearrange("(q p) -> q p", p=P))
                btp = ps_bat.tile([P, R], F32)
                nc.tensor.transpose(btp[:], bflat[:], ident[0:R, 0:R])
                nc.vector.tensor_copy(bat_sb[:], btp[:])

            # ---- stage 1: K/V/Q projections off x^T ----
            with tc.tile_pool(name="ps_qt", bufs=max(SC, 1), space="PSUM") as ps_qt, \
                 tc.tile_pool(name="ps_kv", bufs=1, space="PSUM") as ps_kv:
                kv_full = ps_kv.tile([P, 512], F32)
                kv_ps = kv_full[:, : R * b_loc * 2 * D].rearrange(
                    "p (q b e) -> p q b e", q=R, b=b_loc)
                qt_ps = [ps_qt.tile([P, 512], F32, tag="qt", name="qt") for _ in range(SC)]
                for scq in range(SC):
                    nc.vector.memset(qt_ps[scq][:], 0.0)
                for b in range(b_loc):
                    for q in range(R):
                        for hc in range(HC):
                            nc.tensor.matmul(
                                kv_ps[:, q, b, :],
                                xT[:, hc, b, q * P : (q + 1) * P],
                                wkv_sb[:, hc, :],
                                start=(hc == 0),
                                stop=(hc == HC - 1),
                            )
                    for scq in range(SC):
                        for hc in range(HC):
                            nc.tensor.matmul(
                                qt_ps[scq][32 * b : 32 * b + D, :],
                                wq_sb[:, hc, :],
                                xT[:, hc, b, 512 * scq : 512 * (scq + 1)],
                                start=(hc == 0),
                                stop=(hc == HC - 1),
                                skip_group_check=True,
                                tile_position=(0, 32 * b),
                            )
                nc.vector.tensor_copy(
                    k_pad[:].rearrange("p q (b e) -> p q b e", e=32)[:, :, 0:b_loc, 0:D],
                    kv_ps[:, :, :, 0:D],
                )
                nc.vector.tensor_copy(
                    v_sb[:, :, :, 0:D],
                    kv_ps[:, :, :, D : 2 * D].rearrange("p q b d -> p b q d"),
                )
                for scq in range(SC):
                    nc.vector.tensor_copy(
                        qt_sb[:, 512 * scq : 512 * (scq + 1)], qt_ps[scq][:]
                    )

            # ---- stage 2: M^T = K^T W_at (contraction over s-chunks q) ----
            with tc.tile_pool(name="ps_m", bufs=2, space="PSUM") as ps_m:
                for uc in range(SC):
                    m_ps = ps_m.tile([P, 512], F32, tag="m", name="m")
                    for q in range(R):
                        nc.tensor.matmul(
                            m_ps[:],
                            k_pad[:, q, :],
                            wg[:, q, 512 * uc : 512 * (uc + 1)],
                            start=(q == 0),
                            stop=(q == R - 1),
                        )
                    nc.vector.tensor_copy(
                        m_sb[:, 512 * uc : 512 * (uc + 1)], m_ps[:])

            # ---- stage 3: attend; PE(l,y) pipelined against ACT(exp) ----
            BH = 2                      # batches per l/e buffer
            NB = b_loc // BH            # buffers consumed per (rt, sc)
            with tc.tile_pool(name="esb", bufs=2) as e_pool, \
                 tc.tile_pool(name="ps_y", bufs=max(SC, 1), space="PSUM") as ps_y, \
                 tc.tile_pool(name="ps_l", bufs=2, space="PSUM") as ps_l:
                y_ps = [ps_y.tile([P, 512], F32, tag="y", name="y") for _ in range(SC)]
                for sc in range(SC):
                    nc.vector.memset(y_ps[sc][:], 0.0)
                le_tiles = [
                    (ps_l.tile([P, BH, 512], F32, tag="l", name="l"),
                     e_pool.tile([P, BH, 512], BF16, tag="e", name="e"))
                    for _ in range(2)
                ]
                step = 0
                for rt in range(R):
                    for sc in range(SC):
                        for nb in range(NB):
                            l_ps, e_sb = le_tiles[step % 2]
                            step += 1
                            for j in range(BH):
                                i = nb * BH + j
                                nc.tensor.matmul(
                                    l_ps[:, j, :],
                                    m_sb[32 * i : 32 * i + D, rt * P : (rt + 1) * P],
                                    qt_sb[32 * i : 32 * i + D,
                                          512 * sc : 512 * (sc + 1)],
                                    start=True,
                                    stop=True,
                                    tile_position=(32 * i, 0),
                                )
                            nc.scalar.activation(
                                e_sb[:], l_ps[:], EXP,
                                bias=bat_sb[:, rt : rt + 1], scale=1.0,
                            )
                            for j in range(BH):
                                i = nb * BH + j
                                nc.tensor.matmul(
                                    y_ps[sc][32 * i : 32 * i + D + 1, :],
                                    v_sb[:, i, rt, :],
                                    e_sb[:, j, :],
                                    start=(rt == 0),
                                    stop=(rt == R - 1),
                                    skip_group_check=True,
                                    tile_position=(0, 32 * i),
                                )
                for sc in range(SC):
                    nc.vector.tensor_copy(
                        yt_sb[:, 512 * sc : 512 * (sc + 1)], y_ps[sc][:]
                    )

            # ---- epilogue: classifier + softmax + mean over s ----
            with tc.tile_pool(name="ep", bufs=2) as ep, \
                 tc.tile_pool(name="ps_z", bufs=2, space="PSUM") as ps_z, \
                 tc.tile_pool(name="ps_o", bufs=1, space="PSUM") as ps_o:
                out_ps = ps_o.tile([1, 512], F32)
                KR = next(kr for kr in (16, 8, 4, 2, 1)
                          if kr * b_loc * E <= 512 and R % kr == 0)
                for kh in range(R // KR):
                    z_full = ps_z.tile([P, 512], F32, tag="zf", name="zf")
                    z_ps = z_full[:, : KR * b_loc * E].rearrange(
                        "p (k i e) -> p k i e", k=KR, i=b_loc)
                    for kk in range(KR):
                        k = kh * KR + kk
                        nc.tensor.matmul(
                            z_ps[:, kk, :, :].rearrange("p i e -> p (i e)"),
                            yt_sb[:, k * P : (k + 1) * P],
                            wce[:],
                            start=True,
                            stop=True,
                        )
                    r_sb = ep.tile([P, KR * b_loc], F32, tag="r", name="r")
                    nc.vector.reciprocal(r_sb[:], z_ps[:, :, :, C])
                    zz = ep.tile([P, KR, b_loc, C], F32, tag="zz", name="zz")
                    nc.vector.tensor_tensor(
                        zz[:],
                        z_ps[:, :, :, 0:C],
                        r_sb[:].rearrange("p (k i) -> p k i", k=KR)
                            .unsqueeze(-1).broadcast_to([P, KR, b_loc, C]),
                        mybir.AluOpType.mult,
                    )
                    ez = ep.tile([P, KR, b_loc, C], F32, tag="ez", name="ez")
                    nc.scalar.activation(ez[:], zz[:], EXP)
                    sz = ep.tile([P, KR * b_loc], F32, tag="sz", name="sz")
                    nc.vector.tensor_reduce(
                        sz[:], ez[:], axis=mybir.AxisListType.X, op=mybir.AluOpType.add
                    )
                    rz = ep.tile([P, KR * b_loc], F32, tag="rz", name="rz")
                    nc.vector.reciprocal(rz[:], sz[:])
                    pz = ep.tile([P, KR, b_loc, C], F32, tag="pz", name="pz")
                    nc.vector.tensor_tensor(
                        pz[:],
                        ez[:],
                        rz[:].rearrange("p (k i) -> p k i", k=KR)
                            .unsqueeze(-1).broadcast_to([P, KR, b_loc, C]),
                        mybir.AluOpType.mult,
                    )
                    pc_sb = ep.tile([P, b_loc, C], F32, tag="pc", name="pc")
                    nc.vector.tensor_reduce(
                        pc_sb[:],
                        pz[:].rearrange("p k i c -> p i c k"),
                        axis=mybir.AxisListType.X,
                        op=mybir.AluOpType.add,
                    )
                    nc.tensor.matmul(
                        out_ps[:, : b_loc * C],
                        ones_col[:],
                        pc_sb[:].rearrange("p i c -> p (i c)"),
                        start=(kh == 0),
                        stop=(kh == R // KR - 1),
                    )
                out_sb = ep.tile([1, b_loc * C], F32, tag="o", name="o")
                nc.scalar.mul(out_sb[:], out_ps[:, : b_loc * C], 1.0 / s)
                nc.sync.dma_start(out_t.ap().rearrange("b c -> (b c)")[None, :], out_sb[:])

    nc.finalize()
    return nc


_NC_CACHE = {}


def _get_nc(key=(B_LOC, S_FULL, H_FULL), reps=1):
    if (key, reps) not in _NC_CACHE:
        _NC_CACHE[(key, reps)] = build_nc(*key, reps=reps)
    return _NC_CACHE[(key, reps)]


def _cast_inputs(x, wq, wk, wv, w_at, b_at, w_cls, b_cls):
    import ml_dtypes

    bf16 = ml_dtypes.bfloat16
    x_bf = np.ascontiguousarray(np.asarray(x, dtype=np.float32).astype(bf16))
    shared = {
        "wq": np.asarray(wq, np.float32),
        "wk": np.asarray(wk, np.float32),
        "wv": np.asarray(wv, np.float32),
        "w_at": np.ascontiguousarray(np.asarray(w_at, np.float32).astype(bf16)),
        "b_at": np.asarray(b_at, np.float32),
        "w_cls": np.asarray(w_cls, np.float32),
        "b_cls": np.asarray(b_cls, np.float32),
    }
    return x_bf, shared


def kernel(x, wq, wk, wv, w_at, b_at, w_cls, b_cls):
    from concourse.bass_utils import run_bass_kernel_spmd

    x_bf, shared = _cast_inputs(x, wq, wk, wv, w_at, b_at, w_cls, b_cls)
    nc = _get_nc()
    in_maps = [
        {"xs": x_bf[c * B_LOC : (c + 1) * B_LOC], **shared} for c in range(N_CORES)
    ]
    last_err = None
    for _attempt in range(3):
        try:
            res = run_bass_kernel_spmd(nc, in_maps, list(range(N_CORES))).results
            return np.concatenate([res[c]["out"] for c in range(N_CORES)], axis=0)
        except Exception as e:  # transient NRT/axon execution failures
            last_err = e
    raise last_err


# revision 7
# speedup vs baseline: 1.2915x; 1.2915x over previous
"""Trainium2 Bass kernel for nn_FLinemodel_37185826849029 (bf16, overlapped, early bias/q0/v, deeper e pipeline).

Model (per batch b):
  Q = x@wq, K = x@wk, V = x@wv          [S,4]
  L = (Q K^T) @ W_at + b_at             [S,S]   <- rewritten as Q @ (K^T W_at)
  A = softmax(L, axis=-1)
  y = A @ V                             [S,4]
  p = softmax(y @ w_cls + b_cls)        [S,10]
  out = mean_s p                        [10]

vs the fp32 baseline:
  - x and w_at are cast to bf16 on the host: all large matmuls run at
    1 cyc/row instead of 4, and HBM traffic halves.
  - x^T is produced by two xbar DMA-transposes straight from DRAM
    (2-byte dtype makes this legal), deleting the PE-transpose +
    PSUM-copy machinery of stage 1.
  - natural s/u ordering everywhere (s = 128*q + p); W_at loads as
    [p, q, u] strided rows.
  - stage 3 pipelines PE (l, y matmuls) against ACT (exp) with two
    half-batch l/e buffers; PSUM = 4 banks y + 2*2 banks l.
Sharding: data-parallel over batch, 4 batches per core.
"""

from contextlib import ExitStack

import numpy as np

import concourse.bacc as bacc
import concourse.mybir as mybir
import concourse.tile as tile
from concourse import masks

F32 = mybir.dt.float32
BF16 = mybir.dt.bfloat16
EXP = mybir.ActivationFunctionType.Exp
P = 128

B, S_FULL, H_FULL, D, C = 32, 2048, 256, 4, 10
N_CORES = 8
B_LOC = B // N_CORES


def build_nc(b_loc=B_LOC, s=S_FULL, h=H_FULL, reps=1, dbg=None):
    HC = h // P            # h chunks (2)
    R = s // P             # u tiles of 128 (16)
    SC = s // 512          # 512-wide s blocks (4)
    E = C + 1
    assert HC == 2 and s % 512 == 0

    nc = bacc.Bacc("TRN2", debug=False, target_bir_lowering=False)

    xs_t = nc.dram_tensor("xs", [b_loc, s, h], BF16, kind="ExternalInput")
    wq_t = nc.dram_tensor("wq", [h, D], F32, kind="ExternalInput")
    wk_t = nc.dram_tensor("wk", [h, D], F32, kind="ExternalInput")
    wv_t = nc.dram_tensor("wv", [h, D], F32, kind="ExternalInput")
    wat_t = nc.dram_tensor("w_at", [s, s], BF16, kind="ExternalInput")
    bat_t = nc.dram_tensor("b_at", [s], F32, kind="ExternalInput")
    wcls_t = nc.dram_tensor("w_cls", [D, C], F32, kind="ExternalInput")
    bcls_t = nc.dram_tensor("b_cls", [C], F32, kind="ExternalInput")
    out_t = nc.dram_tensor("out", [b_loc, C], F32, kind="ExternalOutput")

    xs, wat = xs_t.ap(), wat_t.ap()

    with ExitStack() as ctx:
        tc = ctx.enter_context(tile.TileContext(nc))
        const = ctx.enter_context(tc.tile_pool(name="const", bufs=1))
        big = ctx.enter_context(tc.tile_pool(name="big", bufs=1))

        ident = const.tile([P, P], F32)
        masks.make_identity(nc, ident[:])
        ones_col = const.tile([P, 1], F32)
        nc.vector.memset(ones_col[:], 1.0)
        mean_col = const.tile([P, 1], F32)
        nc.vector.memset(mean_col[:], 1.0 / s)

        # w{q,kv}_sb[p, hc, d] = w[hc*P + p, d]; loaded flat (1 descriptor each)
        # and redistributed across partitions with tiny PE transposes.
        wq_sb = const.tile([P, HC, D], BF16)
        wkv_sb = const.tile([P, HC, 2 * D], BF16)
        wflat = const.tile([1, 3, h * D], F32)
        nc.sync.dma_start(wflat[:, 0], wq_t.ap().rearrange("h d -> (h d)")[None, :])
        nc.sync.dma_start(wflat[:, 1], wk_t.ap().rearrange("h d -> (h d)")[None, :])
        nc.sync.dma_start(wflat[:, 2], wv_t.ap().rearrange("h d -> (h d)")[None, :])

        # classifier weights extended: rows (d0..d3, bias), cols (c0..c9, unit),
        # replicated at partition strips 32b for the row-tiled z matmuls.
        wce = const.tile([P, b_loc * E], F32)
        nc.vector.memset(wce[:], 0.0)
        for b in range(b_loc):
            nc.sync.dma_start(wce[32 * b : 32 * b + D, E * b : E * b + C], wcls_t.ap())
            nc.sync.dma_start(
                wce[32 * b + D : 32 * b + D + 1, E * b : E * b + C], bcls_t.ap()[None, :])
            # unit entry at (row D, col E*b + C) of strip b
            nc.gpsimd.affine_select(
                out=wce[32 * b : 32 * b + 32, :],
                in_=wce[32 * b : 32 * b + 32, :],
                pattern=[[1, b_loc * E]],
                compare_op=mybir.AluOpType.not_equal,
                fill=1.0,
                base=-(b_loc * E * D + E * b + C),
                channel_multiplier=b_loc * E,
            )

        wce_bf = const.tile([P, b_loc * E], BF16)

        with tc.tile_pool(name="ps_w", bufs=1, space="PSUM") as ps_w:
            wtp = ps_w.tile([P, 3 * HC * D], F32)
            wfv = wflat[:].rearrange("o w (hh hl d) -> o w hh d hl", hh=HC, d=D)
            for w in range(3):
                for hc in range(HC):
                    for d in range(D):
                        nc.tensor.transpose(
                            wtp[:, (w * HC + hc) * D + d : (w * HC + hc) * D + d + 1],
                            wfv[:, w, hc, d, :],
                            ident[0:1, 0:1],
                        )
            nc.vector.tensor_copy(
                wq_sb[:], wtp[:, : HC * D].rearrange("p (hh d) -> p hh d", d=D))
            nc.vector.tensor_copy(
                wkv_sb[:].rearrange("p hh (w d) -> p w hh d", d=D),
                wtp[:, HC * D :].rearrange("p (w hh d) -> p w hh d", hh=HC, d=D),
            )
        nc.vector.tensor_copy(wce_bf[:], wce[:])

        xT = big.tile([P, HC, b_loc, s], BF16)      # x^T: [h-half, b, s] natural s
        wg = big.tile([P, R, s], BF16)              # W_at[128q+p, u]
        bat_sb = big.tile([P, R], F32)              # b_at[128rt + p]
        qt_sb = big.tile([P, s], BF16)              # rows 32b..: Q_b^T [4, s]
        k_pad = big.tile([P, R, P], BF16)           # col 32b+d = K_b[128q+p, d], else 0
        v_sb = big.tile([P, b_loc, R, D + 1], BF16)  # V[128rt+p, d] + ones col
        m_sb = big.tile([P, s], BF16)               # rows 32b..: M^T, col u natural
        yt_sb = big.tile([P, s], BF16)              # rows 32b..: [yhat^T; rowsum]

        nc.vector.memset(v_sb[:], 1.0)
        nc.vector.memset(k_pad[:], 0.0)

        if dbg is not None:
            dbg.update(xT=xT, wg=wg, bat_sb=bat_sb, qt_sb=qt_sb, k_pad=k_pad,
                       v_sb=v_sb, m_sb=m_sb, yt_sb=yt_sb, wq_sb=wq_sb,
                       wkv_sb=wkv_sb, wce=wce)

        wat_v = wat.rearrange("(q p) u -> p q u", p=P)

        for _rep in range(reps):
            # ---- stage 0: input DMAs; b_at first (the first exp's bias must
            #      not queue behind ~12MB of W_at/x on the FIFO DMA ring) ----
            with tc.tile_pool(name="batld", bufs=1) as batld, \
                 tc.tile_pool(name="ps_bat", bufs=1, space="PSUM") as ps_bat:
                bflat = batld.tile([R, P], F32)
                nc.sync.dma_start(bflat[:], bat_t.ap().rearrange("(q p) -> q p", p=P))
                btp = ps_bat.tile([P, R], F32)
                nc.tensor.transpose(btp[:], bflat[:], ident[0:R, 0:R])
                nc.vector.tensor_copy(bat_sb[:], btp[:])
            nc.sync.dma_start(wg[:, :, 0:512], wat_v[:, :, 0:512])
            for b in range(b_loc):
                for hc in range(HC):
                    nc.sync.dma_start(
                        xT[:, hc, b],
                        xs[b][:, hc * P : (hc + 1) * P],
                        transpose=True,
                    )
            for uc in range(1, SC):
                nc.sync.dma_start(
                    wg[:, :, 512 * uc : 512 * (uc + 1)],
                    wat_v[:, :, 512 * uc : 512 * (uc + 1)],
                )

            # ---- stage 1: K/V/Q projections; m(uc=0) accumulated in-loop so
            #      the first attend bank is ready right after the last K chunk ----
            with tc.tile_pool(name="ps_qt", bufs=max(SC, 1), space="PSUM") as ps_qt, \
                 tc.tile_pool(name="ps_kv", bufs=1, space="PSUM") as ps_kv, \
                 tc.tile_pool(name="ps_m0", bufs=1, space="PSUM") as ps_m0:
                kv_full = ps_kv.tile([P, 512], F32)
                kv_ps = kv_full[:, : R * b_loc * 2 * D].rearrange(
                    "p (q b e) -> p q b e", q=R, b=b_loc)
                m0_ps = ps_m0.tile([P, 512], F32)
                qt_ps = [ps_qt.tile([P, 512], F32, tag="qt", name="qt") for _ in range(SC)]
                for scq in range(SC):
                    nc.vector.memset(qt_ps[scq][:], 0.0)
                def m0_step(q):
                    nc.tensor.matmul(
                        m0_ps[:],
                        k_pad[:, q, :],
                        wg[:, q, 0:512],
                        start=(q == 0),
                        stop=(q == R - 1),
                    )

                for q in range(R):
                    for b in range(b_loc):
                        for hc in range(HC):
                            nc.tensor.matmul(
                                kv_ps[:, q, b, :],
                                xT[:, hc, b, q * P : (q + 1) * P],
                                wkv_sb[:, hc, :],
                                start=(hc == 0),
                                stop=(hc == HC - 1),
                            )
                    nc.vector.tensor_copy(
                        k_pad[:, q].rearrange("p (b e) -> p b e", e=32)[:, 0:b_loc, 0:D],
                        kv_ps[:, q, :, 0:D],
                    )
                    if q >= 1:
                        m0_step(q - 1)
                m0_step(R - 1)
                nc.vector.tensor_copy(m_sb[:, 0:512], m0_ps[:])
                nc.vector.tensor_copy(
                    v_sb[:, :, :, 0:D],
                    kv_ps[:, :, :, D : 2 * D].rearrange("p q b d -> p b q d"),
                )
                for scq in range(SC):
                    for b in range(b_loc):
                        for hc in range(HC):
                            nc.tensor.matmul(
                                qt_ps[scq][32 * b : 32 * b + D, :],
                                wq_sb[:, hc, :],
                                xT[:, hc, b, 512 * scq : 512 * (scq + 1)],
                                start=(hc == 0),
                                stop=(hc == HC - 1),
                                skip_group_check=True,
                                tile_position=(0, 32 * b),
                            )
                    nc.vector.tensor_copy(
                        qt_sb[:, 512 * scq : 512 * (scq + 1)], qt_ps[scq][:]
                    )

            # ---- stages 2+3: attend in two s-halves; stage2 (M^T = K^T W_at)
            #      interleaved per u-bank under half 0's exp stream; epilogue
            #      half 0 overlaps s-half 1 ----
            BH = 2                      # batches per l/e buffer
            NB = b_loc // BH            # buffers consumed per (rt, sc)
            KR = 8                      # yt 128-col blocks per epilogue half
            with tc.tile_pool(name="esb", bufs=3) as e_pool, \
                 tc.tile_pool(name="ps_y", bufs=1, space="PSUM") as ps_y, \
                 tc.tile_pool(name="ps_l", bufs=2, space="PSUM") as ps_l:
                l_tiles = [ps_l.tile([P, BH, 512], F32, tag="l", name="l")
                           for _ in range(2)]
                e_tiles = [e_pool.tile([P, BH, 512], BF16, tag="e", name="e")
                           for _ in range(3)]
                step = [0]

                def attend(rt, y_tiles, scs):
                    for sci, sc in enumerate(scs):
                        for nb in range(NB):
                            l_ps = l_tiles[step[0] % 2]
                            e_sb = e_tiles[step[0] % 3]
                            step[0] += 1
                            for j in range(BH):
                                i = nb * BH + j
                                nc.tensor.matmul(
                                    l_ps[:, j, :],
                                    m_sb[32 * i : 32 * i + D, rt * P : (rt + 1) * P],
                                    qt_sb[32 * i : 32 * i + D,
                                          512 * sc : 512 * (sc + 1)],
                                    start=True,
                                    stop=True,
                                    tile_position=(32 * i, 0),
                                )
                            nc.scalar.activation(
                                e_sb[:], l_ps[:], EXP,
                                bias=bat_sb[:, rt : rt + 1], scale=1.0,
                            )
                            for j in range(BH):
                                i = nb * BH + j
                                nc.tensor.matmul(
                                    y_tiles[sci][32 * i : 32 * i + D + 1, :],
                                    v_sb[:, i, rt, :],
                                    e_sb[:, j, :],
                                    start=(rt == 0),
                                    stop=(rt == R - 1),
                                    skip_group_check=True,
                                    tile_position=(0, 32 * i),
                                )

                # s-half 0 (sc 0,1), stage 2 interleaved per u-bank
                y_tiles = [ps_y.tile([P, 512], F32, tag=f"y{j}", name="y")
                           for j in range(2)]
                for j in range(2):
                    nc.vector.memset(y_tiles[j][:], 0.0)
                with tc.tile_pool(name="ps_m", bufs=2, space="PSUM") as ps_m:
                    for uc in range(SC):
                        if uc > 0:
                            m_ps = ps_m.tile([P, 512], F32, tag="m", name="m")
                            for q in range(R):
                                nc.tensor.matmul(
                                    m_ps[:],
                                    k_pad[:, q, :],
                                    wg[:, q, 512 * uc : 512 * (uc + 1)],
                                    start=(q == 0),
                                    stop=(q == R - 1),
                                )
                            nc.vector.tensor_copy(
                                m_sb[:, 512 * uc : 512 * (uc + 1)], m_ps[:])
                        for rt_in in range(R // SC):
                            attend(uc * (R // SC) + rt_in, y_tiles, (0, 1))
                for j in range(2):
                    nc.vector.tensor_copy(
                        yt_sb[:, 512 * j : 512 * (j + 1)], y_tiles[j][:])

                with tc.tile_pool(name="ep", bufs=2) as ep, \
                     tc.tile_pool(name="ps_z", bufs=1, space="PSUM") as ps_z, \
                     tc.tile_pool(name="ps_o", bufs=1, space="PSUM") as ps_o:
                    out_ps = ps_o.tile([1, 512], F32)

                    def epi_half(kh):
                        z_full = ps_z.tile([P, 512], F32, tag="zf", name="zf")
                        z_ps = z_full[:, : KR * b_loc * E].rearrange(
                            "p (k i e) -> p k i e", k=KR, i=b_loc)
                        for kk in range(KR):
                            k = kh * KR + kk
                            nc.tensor.matmul(
                                z_ps[:, kk, :, :].rearrange("p i e -> p (i e)"),
                                yt_sb[:, k * P : (k + 1) * P],
                                wce_bf[:],
                                start=True,
                                stop=True,
                            )
                        r_sb = ep.tile([P, KR * b_loc], F32, tag="r", name="r")
                        nc.vector.reciprocal(r_sb[:], z_ps[:, :, :, C])
                        zz = ep.tile([P, KR, b_loc, C], F32, tag="zz", name="zz")
                        nc.vector.tensor_tensor(
                            zz[:],
                            z_ps[:, :, :, 0:C],
                            r_sb[:].rearrange("p (k i) -> p k i", k=KR)
                                .unsqueeze(-1).broadcast_to([P, KR, b_loc, C]),
                            mybir.AluOpType.mult,
                        )
                        ez = ep.tile([P, KR, b_loc, C], F32, tag="ez", name="ez")
                        nc.scalar.activation(ez[:], zz[:], EXP)
                        sz = ep.tile([P, KR * b_loc], F32, tag="sz", name="sz")
                        nc.vector.tensor_reduce(
                            sz[:], ez[:], axis=mybir.AxisListType.X,
                            op=mybir.AluOpType.add,
                        )
                        rz = ep.tile([P, KR * b_loc], F32, tag="rz", name="rz")
                        nc.vector.reciprocal(rz[:], sz[:])
                        pz = ep.tile([P, KR, b_loc, C], F32, tag="pz", name="pz")
                        nc.vector.tensor_tensor(
                            pz[:],
                            ez[:],
                            rz[:].rearrange("p (k i) -> p k i", k=KR)
                                .unsqueeze(-1).broadcast_to([P, KR, b_loc, C]),
                            mybir.AluOpType.mult,
                        )
                        pc_sb = ep.tile([P, b_loc, C], F32, tag="pc", name="pc")
                        nc.vector.tensor_reduce(
                            pc_sb[:],
                            pz[:].rearrange("p k i c -> p i c k"),
                            axis=mybir.AxisListType.X,
                            op=mybir.AluOpType.add,
                        )
                        nc.tensor.matmul(
                            out_ps[:, : b_loc * C],
                            mean_col[:],
                            pc_sb[:].rearrange("p i c -> p (i c)"),
                            start=(kh == 0),
                            stop=(kh == 1),
                        )

                    epi_half(0)

                    # s-half 1 (sc 2,3); epilogue half 0 overlaps on ACT/DVE
                    y_tiles = [ps_y.tile([P, 512], F32, tag=f"y{j}", name="y")
                               for j in range(2)]
                    for j in range(2):
                        nc.vector.memset(y_tiles[j][:], 0.0)
                    for rt in range(R):
                        attend(rt, y_tiles, (2, 3))
                    for j in range(2):
                        nc.vector.tensor_copy(
                            yt_sb[:, 512 * (2 + j) : 512 * (3 + j)], y_tiles[j][:])

                    epi_half(1)
                    out_sb = ep.tile([1, b_loc * C], F32, tag="o", name="o")
                    nc.vector.tensor_copy(out_sb[:], out_ps[:, : b_loc * C])
                    nc.sync.dma_start(
                        out_t.ap().rearrange("b c -> (b c)")[None, :], out_sb[:])

    nc.finalize()
    return nc


_NC_CACHE = {}


def _get_nc(key=(B_LOC, S_FULL, H_FULL), reps=1):
    if (key, reps) not in _NC_CACHE:
        _NC_CACHE[(key, reps)] = build_nc(*key, reps=reps)
    return _NC_CACHE[(key, reps)]


def _cast_inputs(x, wq, wk, wv, w_at, b_at, w_cls, b_cls):
    import ml_dtypes

    bf16 = ml_dtypes.bfloat16
    x_bf = np.ascontiguousarray(np.asarray(x, dtype=np.float32).astype(bf16))
    shared = {
        "wq": np.asarray(wq, np.float32),
        "wk": np.asarray(wk, np.float32),
        "wv": np.asarray(wv, np.float32),
        "w_at": np.ascontiguousarray(np.asarray(w_at, np.float32).astype(bf16)),
        "b_at": np.asarray(b_at, np.float32),
        "w_cls": np.asarray(w_cls, np.float32),
        "b_cls": np.asarray(b_cls, np.float32),
    }
    return x_bf, shared


def kernel(x, wq, wk, wv, w_at, b_at, w_cls, b_cls):
    from concourse.bass_utils import run_bass_kernel_spmd

    x_bf, shared = _cast_inputs(x, wq, wk, wv, w_at, b_at, w_cls, b_cls)
    nc = _get_nc()
    in_maps = [
        {"xs": x_bf[c * B_LOC : (c + 1) * B_LOC], **shared} for c in range(N_CORES)
    ]
    last_err = None
    for _attempt in range(3):
        try:
            res = run_bass_kernel_spmd(nc, in_maps, list(range(N_CORES))).results
            return np.concatenate([res[c]["out"] for c in range(N_CORES)], axis=0)
        except Exception as e:  # transient NRT/axon execution failures
            last_err = e
    raise last_err
